# revision 16
# baseline (speedup 1.0000x reference)
"""Causal multi-head self-attention with RoPE on 8 Trainium2 NeuronCores.

Problem: B=2, S=2048, D=2048, 16 heads x head_dim 128, causal mask, RoPE.

Sharding (data + tensor parallel per the hint): 8 cores = 2 batch rows x 4
head-groups (4 heads each). Each core computes, for its batch row and its 4
heads: Q/K/V projections, RoPE, causal softmax attention, and the partial
output projection through its head-group's slice of Wo. The host sums the 4
head-group partials per batch row (row-parallel Wo unshard).

Schedule: the kernel is SOFTWARE-PIPELINED across s-quarters -- the
(ACT-paced) attention + out-projection of quarter q-1 is interleaved, at
~instruction granularity, into the (PE-dense) Q/K/V projections of quarter
q, so the tensor engine never idles while the scalar engine runs exps.

Per-core dataflow (all matmul operands bf16; fp32 PSUM):
  - x^T, weights and the output live in DRAM pre-swizzled to SBUF layout
    [128, *] (host does the transposes), so every DMA is a large contiguous
    2D copy and weights stay RESIDENT in SBUF across all quarters.
  - QT/KT computed as [hd, s]; ACT drains each projection PSUM to SBUF bf16
    (freeing the bank), then RoPE runs on DVE at 16-bit rate: the head dim
    is host-permuted so each RoPE pair partner sits +-16 partitions away
    within one 32-lane quadrant (stream_shuffle + mul/add; the permutation
    cancels in Q.K).
  - scores^T [k, q]: two 512-wide score matmuls fill a [128,1024] 2-bank
    PSUM tile; ONE ACT exp converts the pair (scale folded in); causal mask
    via gpsimd affine_select on diagonal subtiles (multiplicative 0).
  - softmax denominator: DVE accumulates lacc[k,q] += w over k-tiles (fp32),
    then a single all-ones [128,128] f32r stationary matmul per (head,
    quarter) partition-sums lacc INTO a pre-broadcast [128,q] PSUM tile;
    1/l is one DVE reciprocal_approx_fast; O^T * (1/l) is one DVE mul.
  - O^T [hd, q] = V_slice.T @ w^T accumulated over k-tiles on the PE.
  - out^T = WoT_slice.T @ O^T, staged PSUM->SBUF bf16 on ACT, DMA'd out.
"""

import math

import numpy as np

import concourse.bass as bass
import concourse.mybir as mybir
import concourse.tile as tile
from concourse import bacc

B = 2
D = 2048
H_LOC = 4  # heads per core
HD = 128  # head dim
QW = 512  # s-quarter width (and matmul moving width)
NQ = 4  # number of s-quarters
DC = D // HD  # contraction chunks
N_CORES = 8
THETA = 10000.0
F32 = mybir.dt.float32
F32R = mybir.dt.float32r
BF16 = mybir.dt.bfloat16


def build_program(S=2048, repeat=1, phases='full'):
    """Build the per-core SPMD Bass program (all 8 cores run this).

    repeat>1 re-runs the whole computation serially inside one NEFF
    (including the resident-weight DMAs, for honest per-exec timing).
    """
    assert S == NQ * QW
    scale = 1.0 / math.sqrt(HD)

    nc = bacc.Bacc("TRN2", target_bir_lowering=False, debug=False, num_devices=N_CORES)
    # DRAM layouts are host-swizzled to match SBUF: [128 partitions, free].
    xt_d = nc.dram_tensor("xt", [HD, NQ * DC * QW], BF16, kind="ExternalInput").ap()
    wqt_d = nc.dram_tensor("wqt", [HD, DC * QW], BF16, kind="ExternalInput").ap()
    wkt_d = nc.dram_tensor("wkt", [HD, DC * QW], BF16, kind="ExternalInput").ap()
    wvt_d = nc.dram_tensor("wvt", [HD, DC * QW], BF16, kind="ExternalInput").ap()
    wot_d = nc.dram_tensor("wot", [HD, H_LOC * D], BF16, kind="ExternalInput").ap()
    cos2_d = nc.dram_tensor("cos2", [HD, S], BF16, kind="ExternalInput").ap()
    sinpm_d = nc.dram_tensor("sinpm", [HD, S], BF16, kind="ExternalInput").ap()
    ones_d = nc.dram_tensor("ones", [HD, HD], F32R, kind="ExternalInput").ap()
    outt_d = nc.dram_tensor("outt", [HD, NQ * 16 * QW], BF16, kind="ExternalOutput").ap()

    with tile.TileContext(nc) as tc:
        with (
            tc.tile_pool(name="const", bufs=1) as constp,
            tc.tile_pool(name="ktv", bufs=1) as ktvp,
            tc.tile_pool(name="xtp", bufs=2) as xtp,
            tc.tile_pool(name="qtp", bufs=6) as qtp,
            tc.tile_pool(name="rope", bufs=4) as rtp,
            tc.tile_pool(name="wexp", bufs=3) as wep,
            tc.tile_pool(name="lac", bufs=3) as lap,
            tc.tile_pool(name="otp", bufs=6) as otp,
            tc.tile_pool(name="bcast", bufs=2) as bcp,
            tc.tile_pool(name="outsb", bufs=2) as outsbp,
            tc.tile_pool(name="ps", bufs=1, space="PSUM") as psp,
        ):
            # Resident weights, one SBUF tile per matrix. Slice (d, h) of
            # Wk/Wq/Wv lives at [:, d*QW + h*HD : d*QW + (h+1)*HD].
            wk_sb = constp.tile([HD, DC * QW], BF16, tag="wk", name="wk_sb")
            wq_sb = constp.tile([HD, DC * QW], BF16, tag="wq", name="wq_sb")
            wv_sb = constp.tile([HD, DC * QW], BF16, tag="wv", name="wv_sb")
            wo_sb = constp.tile([HD, H_LOC * D], BF16, tag="wo", name="wo_sb")
            cos2 = constp.tile([HD, S], BF16, tag="cos2", name="cos2_sb")
            sinpm = constp.tile([HD, S], BF16, tag="sinpm", name="sinpm_sb")
            ones = constp.tile([HD, HD], F32R, tag="ones", name="ones_sb")

            # K per (head, quarter) tile; V per 128-k-tile. Separate tiles so
            # cross-quarter pipelining never creates false subtile conflicts.
            ktq = [
                [ktvp.tile([HD, QW], BF16, tag=f"kt{h}q{qq}", name=f"kt{h}q{qq}")
                 for qq in range(NQ)]
                for h in range(H_LOC)
            ]
            vt = [
                ktvp.tile([HD, QW], BF16, tag=f"v{i}", name=f"v{i}")
                for i in range(S // HD)
            ]

            # pair-partner swap: +-16 within each 32-partition quadrant
            SHUF_MASK = [(i + 16) % 32 for i in range(32)]

            def rope(pre, q, out_ap):
                # out = R(pos) * pre (pre is the bf16 SBUF copy of the
                # projection PSUM). Lane-local thanks to the host-side
                # head-dim permutation; sinpm carries the pair sign.
                sl = slice(q * QW, (q + 1) * QW)
                shuf = rtp.tile([HD, QW], BF16, tag="shuf", name="shuf", bufs=2)
                nc.vector.stream_shuffle(shuf[:], pre[:], SHUF_MASK)
                ta = rtp.tile([HD, QW], BF16, tag="ta", name="ta", bufs=2)
                nc.vector.tensor_mul(ta[:], pre[:], cos2[:, sl])
                tb = rtp.tile([HD, QW], BF16, tag="tb", name="tb", bufs=2)
                nc.vector.tensor_mul(tb[:], shuf[:], sinpm[:, sl])
                nc.vector.tensor_add(out_ap, ta[:], tb[:])

            # qts[(rep, q)] -> list of 4 Q tiles, consumed by the lagged
            # attention stream one body later.
            qts_store = {}

            def proj_stream(rep, q):
                """Projections K -> Q -> V for quarter q. Yields between
                emission steps so attention(q-1) can interleave."""
                sl = slice(q * QW, (q + 1) * QW)
                load_w = q == 0
                x_sb = xtp.tile([HD, DC * QW], BF16, tag="x", name="x_sb")
                if load_w:
                    # first two d-chunks of Wk and x land first so the K
                    # d-loop starts after ~0.5 MB of DMA, not the full 4 MB.
                    nc.sync.dma_start(wk_sb[:, : 2 * QW], wkt_d[:, : 2 * QW])
                    nc.sync.dma_start(x_sb[:, : 2 * QW], xt_d[:, q * DC * QW : (q * DC + 2) * QW])
                    nc.sync.dma_start(wk_sb[:, 2 * QW : 8 * QW], wkt_d[:, 2 * QW : 8 * QW])
                    nc.sync.dma_start(
                        x_sb[:, 2 * QW : 4 * QW],
                        xt_d[:, (q * DC + 2) * QW : (q * DC + 4) * QW],
                    )
                    nc.sync.dma_start(wk_sb[:, 8 * QW :], wkt_d[:, 8 * QW :])
                for i in range(0 if not load_w else 1, 4):
                    csl = slice(i * 4 * QW, (i + 1) * 4 * QW)
                    nc.sync.dma_start(
                        x_sb[:, csl], xt_d[:, (q * DC + i * 4) * QW : (q * DC + (i + 1) * 4) * QW]
                    )
                yield

                qts = []
                qts_store[(rep, q)] = qts  # filled in-place during Q-RoPE
                for which, (w_sb, w_d) in enumerate(
                    ((wk_sb, wkt_d), (wq_sb, wqt_d))
                ):
                    pps = [
                        psp.tile([HD, QW], F32, tag="pa", name=f"pps{h}", bufs=4)
                        for h in range(H_LOC)
                    ]
                    for d in range(DC):
                        if load_w and which == 0 and d == 2:
                            # tables (needed by RoPE at end of this d-loop)
                            nc.sync.dma_start(cos2[:], cos2_d[:])
                            nc.sync.dma_start(sinpm[:], sinpm_d[:])
                            nc.sync.dma_start(ones[:], ones_d[:])
                        if load_w and which == 0 and d == 6:
                            nc.sync.dma_start(wq_sb[:, : 8 * QW], wqt_d[:, : 8 * QW])
                        if load_w and which == 0 and d == 10:
                            nc.sync.dma_start(wq_sb[:, 8 * QW :], wqt_d[:, 8 * QW :])
                        if load_w and which == 1 and d == 4:
                            nc.sync.dma_start(wv_sb[:, : 8 * QW], wvt_d[:, : 8 * QW])
                        if load_w and which == 1 and d == 8:
                            nc.sync.dma_start(wv_sb[:, 8 * QW :], wvt_d[:, 8 * QW :])
                        if load_w and which == 1 and d == 12:
                            nc.sync.dma_start(wo_sb[:], wot_d[:])
                        for h in range(H_LOC):
                            nc.tensor.matmul(
                                pps[h][:],
                                w_sb[:, d * QW + h * HD : d * QW + (h + 1) * HD],
                                x_sb[:, d * QW : (d + 1) * QW],
                                start=(d == 0),
                                stop=(d == DC - 1),
                            )
                        if d % 4 == 3:
                            yield
                    for h in range(H_LOC):
                        # ACT drains the PSUM bank quickly; RoPE then runs
                        # all-bf16 on DVE at 16-bit rate.
                        pre = rtp.tile([HD, QW], BF16, tag="pre", name="pre", bufs=2)
                        nc.scalar.copy(pre[:], pps[h][:])
                        if which == 0:
                            rope(pre, q, ktq[h][q][:])
                        else:
                            q_sb = qtp.tile([HD, QW], BF16, tag="qt", name=f"qt{h}")
                            rope(pre, q, q_sb[:])
                            qts.append(q_sb)
                        yield

                # --- V projection: V[s_tile, e] for this quarter ---
                vps = [
                    psp.tile([HD, QW], F32, tag="pa", name=f"vps{st}", bufs=4)
                    for st in range(4)
                ]
                for d in range(DC):
                    for st in range(4):
                        nc.tensor.matmul(
                            vps[st][:],
                            x_sb[:, d * QW + st * HD : d * QW + (st + 1) * HD],
                            wv_sb[:, d * QW : (d + 1) * QW],
                            start=(d == 0),
                            stop=(d == DC - 1),
                        )
                    if d % 4 == 3:
                        yield
                for st in range(4):
                    nc.scalar.copy(vt[q * 4 + st][:], vps[st][:])

            ots_store = {}
            early_state = {}

            def attn_stream(rep, q, tail=False):
                """Causal attention heads for quarter q (projections of
                quarter q already emitted). Yields between steps so it can
                interleave into the NEXT quarter's projections. The
                out-projection is emitted separately (outproj_emit) because
                its PSUM tiles share the "pa" ring with the projections.

                tail=True marks the drain body (no projections running):
                odd score-pairs then borrow the idle "pa" PSUM banks so the
                score->exp pipeline is double-buffered."""
                qts = qts_store.pop((rep, q))
                nk = (q + 1) * 4  # k-tiles in causal range
                np_ = nk // 2  # exp pairs
                ots = []
                for h in range(H_LOC):
                    ot_ps = psp.tile([HD, QW], F32, tag="otl", name="ot_ps", bufs=2)
                    lacc = None
                    pend = None
                    for p in range(np_):
                        # two 512-wide score matmuls -> one 2-bank PSUM pair
                        # (odd tail pairs instead use two idle "pa" banks)
                        if tail and p % 2 == 1:
                            s2h = [
                                psp.tile([HD, QW], F32, tag="pa", name="s1", bufs=4)
                                for _ in range(2)
                            ]
                        else:
                            s2h = None
                            s2 = psp.tile(
                                [HD, 2 * QW], F32, tag="sp", name="s2", bufs=1
                            )
                        for half in range(2):
                            ki = 2 * p + half
                            kq = ki // 4
                            nc.tensor.matmul(
                                s2h[half][:] if s2h else s2[:, half * QW : (half + 1) * QW],
                                ktq[h][kq][:, (ki % 4) * HD : (ki % 4 + 1) * HD],
                                qts[h][:],
                                start=True,
                                stop=True,
                            )
                        w2 = wep.tile([HD, 2 * QW], BF16, tag="wexp", name="w2")
                        if s2h:
                            for half in range(2):
                                nc.scalar.activation(
                                    w2[:, half * QW : (half + 1) * QW],
                                    s2h[half][:],
                                    mybir.ActivationFunctionType.Exp,
                                    scale=scale,
                                )
                        else:
                            nc.scalar.activation(
                                w2[:],
                                s2[:],
                                mybir.ActivationFunctionType.Exp,
                                scale=scale,
                            )
                        for half in range(2):
                            ki = 2 * p + half
                            if ki >= q * 4:
                                # diagonal: zero w^T where q_glob < k_glob
                                nc.gpsimd.affine_select(
                                    out=w2[:, half * QW : (half + 1) * QW],
                                    in_=w2[:, half * QW : (half + 1) * QW],
                                    compare_op=mybir.AluOpType.is_ge,
                                    fill=0.0,
                                    base=q * QW - ki * HD,
                                    pattern=[[1, QW]],
                                    channel_multiplier=-1,
                                )
                        # denominator accumulation on DVE (fp32)
                        l_new = lap.tile([HD, QW], F32R, tag="lacc", name="lacc")
                        if lacc is None:
                            nc.vector.tensor_add(
                                l_new[:], w2[:, :QW], w2[:, QW:]
                            )
                        else:
                            t = lap.tile([HD, QW], F32R, tag="lt", name="lt", bufs=1)
                            nc.vector.tensor_add(t[:], w2[:, :QW], w2[:, QW:])
                            nc.vector.tensor_add(l_new[:], lacc[:], t[:])
                        lacc = l_new
                        yield

                        if pend is not None:
                            pki, pw2 = pend
                            for half in range(2):
                                ki = 2 * pki + half
                                nc.tensor.matmul(
                                    ot_ps[:],
                                    vt[ki][:, h * HD : (h + 1) * HD],
                                    pw2[:, half * QW : (half + 1) * QW],
                                    start=(ki == 0),
                                    stop=False,
                                )
                            yield
                        pend = (p, w2)

                    pki, pw2 = pend
                    for half in range(2):
                        ki = 2 * pki + half
                        nc.tensor.matmul(
                            ot_ps[:],
                            vt[ki][:, h * HD : (h + 1) * HD],
                            pw2[:, half * QW : (half + 1) * QW],
                            start=(ki == 0),
                            stop=(ki == nk - 1),
                        )
                    # partition-sum of lacc, pre-broadcast across partitions
                    l_ps = psp.tile([HD, QW], F32, tag="otl", name="l_ps", bufs=2)
                    nc.tensor.matmul(
                        l_ps[:], ones[:], lacc[:], start=True, stop=True
                    )
                    rc = bcp.tile([HD, QW], F32, tag="rc", name="rc")
                    nc.vector.reciprocal_approx_fast(rc[:], l_ps[:])
                    ot_sb = otp.tile([HD, QW], BF16, tag="ot", name=f"ot{h}")
                    nc.vector.tensor_mul(ot_sb[:], ot_ps[:], rc[:])
                    ots.append(ot_sb)
                    yield
                ots_store[(rep, q)] = ots

            def attn_early(rep, q):
                """Early (non-diagonal) score pairs of heads 0-1 of the FINAL
                quarter: they touch only quarter <q K/V plus this quarter's
                Q, so they can interleave into the final projection body.
                The O^T psums stay open (stop=False); attn_finish completes
                them. Dedicated lacc tags (l30/l31) carry the denominator
                chains across the body boundary without ring deadlocks."""
                qts = qts_store[(rep, q)]
                for h in (0, 1):
                    while len(qts) <= h:  # Q-RoPE for this head not yet emitted
                        yield
                    ot_ps = psp.tile([HD, QW], F32, tag="otl", name="ot_ps", bufs=2)
                    lacc = None
                    pend = None
                    for p in range(2 * q):
                        s2 = psp.tile([HD, 2 * QW], F32, tag="sp", name="s2", bufs=1)
                        for half in range(2):
                            ki = 2 * p + half
                            nc.tensor.matmul(
                                s2[:, half * QW : (half + 1) * QW],
                                ktq[h][ki // 4][:, (ki % 4) * HD : (ki % 4 + 1) * HD],
                                qts[h][:],
                                start=True,
                                stop=True,
                            )
                        w2 = wep.tile([HD, 2 * QW], BF16, tag="wexp", name="w2")
                        nc.scalar.activation(
                            w2[:], s2[:], mybir.ActivationFunctionType.Exp, scale=scale
                        )
                        l_new = lap.tile([HD, QW], F32R, tag=f"l3{h}", name="lacc", bufs=2)
                        if lacc is None:
                            nc.vector.tensor_add(l_new[:], w2[:, :QW], w2[:, QW:])
                        else:
                            t = lap.tile([HD, QW], F32R, tag="lt", name="lt", bufs=1)
                            nc.vector.tensor_add(t[:], w2[:, :QW], w2[:, QW:])
                            nc.vector.tensor_add(l_new[:], lacc[:], t[:])
                        lacc = l_new
                        yield
                        if pend is not None:
                            for half in range(2):
                                ki = 2 * pend[0] + half
                                nc.tensor.matmul(
                                    ot_ps[:],
                                    vt[ki][:, h * HD : (h + 1) * HD],
                                    pend[1][:, half * QW : (half + 1) * QW],
                                    start=(ki == 0),
                                    stop=False,
                                )
                            yield
                        pend = (p, w2)
                    # consume the last pend so no wexp tile is held across
                    # the body boundary (the psum stays open instead)
                    for half in range(2):
                        ki = 2 * pend[0] + half
                        nc.tensor.matmul(
                            ot_ps[:],
                            vt[ki][:, h * HD : (h + 1) * HD],
                            pend[1][:, half * QW : (half + 1) * QW],
                            start=(ki == 0),
                            stop=False,
                        )
                    early_state[(rep, q, h)] = (ot_ps, lacc)
                    yield

            def attn_finish(rep, q):
                """Drain body: finish heads 0-1 (diagonal pairs + softmax
                normalization) and run heads 2-3 in full, with the tail
                PSUM tricks (odd pairs on idle "pa" banks; resumed heads'
                l_ps on the "sp" ring to avoid an otl WAR cycle)."""
                qts = qts_store.pop((rep, q))
                nk = (q + 1) * 4
                np_ = nk // 2
                ots = []
                for h in range(H_LOC):
                    resumed = h in (0, 1)
                    if resumed:
                        ot_ps, lacc = early_state.pop((rep, q, h))
                        p_start = 2 * q
                        pend = None
                    else:
                        ot_ps = psp.tile([HD, QW], F32, tag="otl", name="ot_ps", bufs=2)
                        lacc = None
                        p_start = 0
                        pend = None
                    for p in range(p_start, np_):
                        if p % 2 == 1:
                            s2h = [
                                psp.tile([HD, QW], F32, tag="pa", name="s1", bufs=4)
                                for _ in range(2)
                            ]
                        else:
                            s2h = None
                            s2 = psp.tile([HD, 2 * QW], F32, tag="sp", name="s2", bufs=1)
                        for half in range(2):
                            ki = 2 * p + half
                            nc.tensor.matmul(
                                s2h[half][:] if s2h else s2[:, half * QW : (half + 1) * QW],
                                ktq[h][ki // 4][:, (ki % 4) * HD : (ki % 4 + 1) * HD],
                                qts[h][:],
                                start=True,
                                stop=True,
                            )
                        w2 = wep.tile([HD, 2 * QW], BF16, tag="wexp", name="w2")
                        if s2h:
                            for half in range(2):
                                nc.scalar.activation(
                                    w2[:, half * QW : (half + 1) * QW],
                                    s2h[half][:],
                                    mybir.ActivationFunctionType.Exp,
                                    scale=scale,
                                )
                        else:
                            nc.scalar.activation(
                                w2[:], s2[:], mybir.ActivationFunctionType.Exp, scale=scale
                            )
                        for half in range(2):
                            ki = 2 * p + half
                            if ki >= q * 4:
                                nc.gpsimd.affine_select(
                                    out=w2[:, half * QW : (half + 1) * QW],
                                    in_=w2[:, half * QW : (half + 1) * QW],
                                    compare_op=mybir.AluOpType.is_ge,
                                    fill=0.0,
                                    base=q * QW - ki * HD,
                                    pattern=[[1, QW]],
                                    channel_multiplier=-1,
                                )
                        ltag = f"l3{h}" if resumed else "lacc"
                        lbufs = 2 if resumed else None
                        l_new = lap.tile([HD, QW], F32R, tag=ltag, name="lacc", bufs=lbufs)
                        if lacc is None:
                            nc.vector.tensor_add(l_new[:], w2[:, :QW], w2[:, QW:])
                        else:
                            t = lap.tile([HD, QW], F32R, tag="lt", name="lt", bufs=1)
                            nc.vector.tensor_add(t[:], w2[:, :QW], w2[:, QW:])
                            nc.vector.tensor_add(l_new[:], lacc[:], t[:])
                        lacc = l_new
                        if pend is not None:
                            for half in range(2):
                                ki = 2 * pend[0] + half
                                nc.tensor.matmul(
                                    ot_ps[:],
                                    vt[ki][:, h * HD : (h + 1) * HD],
                                    pend[1][:, half * QW : (half + 1) * QW],
                                    start=(ki == 0),
                                    stop=False,
                                )
                        pend = (p, w2)
                    for half in range(2):
                        ki = 2 * pend[0] + half
                        nc.tensor.matmul(
                            ot_ps[:],
                            vt[ki][:, h * HD : (h + 1) * HD],
                            pend[1][:, half * QW : (half + 1) * QW],
                            start=(ki == 0),
                            stop=(ki == nk - 1),
                        )
                    l_ps = psp.tile(
                        [HD, QW], F32,
                        tag=("sp" if resumed else "otl"),
                        name="l_ps",
                        bufs=(1 if resumed else 2),
                    )
                    nc.tensor.matmul(l_ps[:], ones[:], lacc[:], start=True, stop=True)
                    rc = bcp.tile([HD, QW], F32, tag="rc", name="rc")
                    nc.vector.reciprocal_approx_fast(rc[:], l_ps[:])
                    ot_sb = otp.tile([HD, QW], BF16, tag="ot", name=f"ot{h}")
                    nc.vector.tensor_mul(ot_sb[:], ot_ps[:], rc[:])
                    ots.append(ot_sb)
                ots_store[(rep, q)] = ots

            def outproj_emit(rep, q):
                # --- partial output projection: out^T[d, q] += WoT.T @ O^T ---
                # Pure PE work (plus ACT staging); emitted un-interleaved at
                # body end, after the projections have released the "pa" ring.
                ots = ots_store.pop((rep, q))
                for gr in range(4):
                    ops_ = [
                        psp.tile([HD, QW], F32, tag="pa", name=f"ops{dt}", bufs=4)
                        for dt in range(4)
                    ]
                    for h in range(H_LOC):
                        for dt in range(4):
                            nc.tensor.matmul(
                                ops_[dt][:],
                                wo_sb[:, h * D + (gr * 4 + dt) * HD : h * D + (gr * 4 + dt + 1) * HD],
                                ots[h][:],
                                start=(h == 0),
                                stop=(h == H_LOC - 1),
                            )
                    o_sb = outsbp.tile([HD, 4 * QW], BF16, tag="osb", name="o_sb", bufs=2)
                    for dt in range(4):
                        nc.scalar.copy(o_sb[:, dt * QW : (dt + 1) * QW], ops_[dt][:])
                    nc.sync.dma_start(
                        outt_d[:, (q * 16 + gr * 4) * QW : (q * 16 + (gr + 1) * 4) * QW],
                        o_sb[:],
                    )

            def interleave(a, steps_a, b, steps_b):
                """Emit a and b alternately, pacing b to finish slightly
                before a runs out, then drain both."""
                ratio = steps_b / max(1, steps_a - 2)
                acc = 0.0
                a_live = True
                b_live = b is not None
                while a_live or b_live:
                    if a_live:
                        try:
                            next(a)
                        except StopIteration:
                            a_live = False
                    acc += ratio
                    while (acc >= 1.0 or not a_live) and b_live:
                        acc -= 1.0
                        try:
                            next(b)
                        except StopIteration:
                            b_live = False

            def n_steps(kind, q):
                # yield counts (mirror the stream structure) for pacing
                if kind == 'proj':
                    return 1 + 2 * (DC // 4) + 2 * H_LOC + DC // 4
                return H_LOC * 2 * ((q + 1) * 2)

            import itertools

            bodies = [(rep, q) for rep in range(repeat) for q in range(NQ)]
            for i, (rep, q) in enumerate(bodies):
                prev = bodies[i - 1] if i > 0 else None
                b, sb = (attn_stream(*prev), n_steps('attn', prev[1])) if prev else (None, 0)
                if i == len(bodies) - 1 and prev:
                    # fold the final quarter's early (off-diagonal) pairs of
                    # heads 0-1 into this body -- they only need quarters <q
                    # K/V and this body's Q projections
                    b = itertools.chain(b, attn_early(rep, q))
                    sb += 2 * (2 * q * 2 + 1)
                interleave(proj_stream(rep, q), n_steps('proj', q), b, sb)
                if prev:
                    outproj_emit(*prev)
            # drain: finish heads 0-1, run heads 2-3, out-projection
            rep, q = bodies[-1]
            attn_finish(rep, q)
            outproj_emit(rep, q)
    nc.compile()
    return nc


def prep_inputs(x, token_positions, Wq, Wk, Wv, Wo):
    """Shard + swizzle the full inputs into 8 per-core input maps.

    All tensors are laid out host-side exactly as their SBUF destinations
    ([128 partitions, free]), so on-device DMAs are contiguous 2D copies.
    """
    import ml_dtypes

    BF = ml_dtypes.bfloat16
    S = x.shape[1]
    x = np.asarray(x, np.float32)
    pos = np.asarray(token_positions).astype(np.float32)
    k = np.arange(HD // 2, dtype=np.float32)
    inv_freq = (1.0 / (THETA ** (2.0 * k / HD))).astype(np.float32)
    freqs = pos[:, None] * inv_freq[None, :]  # [S, 64]
    cos = np.cos(freqs).T.astype(np.float32)  # [64, S]
    sin = np.sin(freqs).T.astype(np.float32)
    # head-dim permutation chosen so each RoPE pair partner sits +-16
    # partitions away within the same 32-partition quadrant (enables the
    # on-device stream_shuffle). Partition n holds:
    #   g, r = divmod(n, 32); j = 16*g + (r % 16)   (frequency index)
    #   original dim 2j   if r < 16 ("even" slot, rotates with -sin)
    #   original dim 2j+1 otherwise ("odd" slot, rotates with +sin)
    n = np.arange(HD)
    g, r = n // 32, n % 32
    j = 16 * g + (r % 16)
    odd = (r >= 16).astype(np.int64)
    perm = 2 * j + odd
    cos2 = np.ascontiguousarray(cos[j]).astype(BF)  # [128, S]
    sinpm = np.ascontiguousarray(np.where(odd[:, None], sin[j], -sin[j])).astype(BF)
    ones = np.ones((HD, HD), np.float32)

    # x^T chunks in SBUF layout: [128, (q*DC + d)*QW + j] = x[b, q*QW+j, d*128+p]
    xts = [
        np.ascontiguousarray(
            x[b].reshape(NQ, QW, DC, HD).transpose(3, 0, 2, 1).reshape(HD, NQ * DC * QW)
        ).astype(BF)
        for b in range(B)
    ]

    in_maps = []
    for c in range(N_CORES):
        b, gidx = c // 4, c % 4
        rows = slice(gidx * H_LOC * HD, (gidx + 1) * H_LOC * HD)

        def permT(W):
            Wg = np.asarray(W, np.float32)[rows]  # [512, D]
            Wg = Wg.reshape(H_LOC, HD, D)[:, perm, :].reshape(H_LOC * HD, D)
            WgT = Wg.T  # [D, 512]
            return np.ascontiguousarray(
                WgT.reshape(DC, HD, H_LOC * HD).transpose(1, 0, 2).reshape(HD, DC * QW)
            ).astype(BF)

        def plainT(W):
            WgT = np.asarray(W, np.float32)[rows].T  # [D, 512]
            return np.ascontiguousarray(
                WgT.reshape(DC, HD, H_LOC * HD).transpose(1, 0, 2).reshape(HD, DC * QW)
            ).astype(BF)

        WoT = np.asarray(Wo, np.float32)[:, rows].T  # [512 e, D]
        wot = np.ascontiguousarray(
            WoT.reshape(H_LOC, HD, D).transpose(1, 0, 2).reshape(HD, H_LOC * D)
        ).astype(BF)

        in_maps.append(
            {
                "xt": xts[b],
                "wqt": permT(Wq),
                "wkt": permT(Wk),
                "wvt": plainT(Wv),
                "wot": wot,
                "cos2": cos2,
                "sinpm": sinpm,
                "ones": ones,
            }
        )
    return in_maps


def combine_outputs(outts):
    """outts: 8 per-core swizzled bf16 partials [128, NQ*16*QW] -> [B, S, D].

    DRAM col (q*16 + dg)*QW + j at partition p holds
    out^T[dg*128+p, q*QW+j] = out[q*QW+j, dg*128+p].
    """
    S = NQ * QW
    full = []
    for b in range(B):
        acc = None
        for o in outts[b * 4 : (b + 1) * 4]:
            part = (
                np.asarray(o, dtype=np.float32)
                .reshape(HD, NQ, 16, QW)
                .transpose(1, 3, 2, 0)
                .reshape(S, D)
            )
            acc = part if acc is None else acc + part
        full.append(acc)
    return np.stack(full)


_NC = None


def _get_nc():
    global _NC
    if _NC is None:
        _NC = build_program()
    return _NC


def kernel(x, token_positions, Wq, Wk, Wv, Wo):
    from concourse.bass_utils import run_bass_kernel_spmd

    nc = _get_nc()
    in_maps = prep_inputs(x, token_positions, Wq, Wk, Wv, Wo)
    res = run_bass_kernel_spmd(nc, in_maps, core_ids=list(range(N_CORES)))
    return combine_outputs([r["outt"] for r in res.results])
